# revision 1
# baseline (speedup 1.0000x reference)
"""MoE router (AutonomousRouter) for TRN2, 8 NeuronCores.

Computes reference:
    act    = einsum('bnd,edc->bnec', x, W)          B,N,D,E,C = 4,2048,2048,8,512
    logits = ||act||_2 over c                       [B,N,E]
    probs  = softmax(logits, -1)
    top-2 routing with capacity 640 (priority = order within k-major (choice, token) sequence)
    out    = stack([dispatch, combine])             [2,B,N,E,640] fp32

Sharding: data-parallel over tokens; core i <- tokens [i*1024, (i+1)*1024) of the
flattened [8192] token axis (= batch b=i//2, half i%2). Weights replicated.

Phase A (device): bf16x2-split matmuls (fp32-grade logits at 3x bf16 rate) ->
  sum-of-squares -> top-2 via max8 on sumsq (monotone in logits, sub-ulp
  lower-index tie-break) -> softmax (ACT sqrt/exp) -> one-hots -> core-local
  exclusive cumsums per choice slot (PE triangular matmuls, exact integer fp32).
Host glue: combines per-core totals into cross-core priority offsets (64 scalars).
Phase B (device): per-(token,choice) one-hot rows (iota==slot)*{1,prob} built on
  DVE and indirect-scattered into the pre-zeroed dense outputs.
"""
import numpy as np

import concourse.bacc as bacc
import concourse.mybir as mybir
from concourse.tile import TileContext
from concourse.bass_utils import run_bass_kernel_spmd

P = 128          # partitions
B, N, D, E, C = 4, 2048, 2048, 8, 512
CAP = 640
NCORES = 8
TOK = (B * N) // NCORES     # tokens per core = 1024
NT = TOK // P               # token tiles per core = 8
KT = D // P                 # contraction tiles = 16

f32 = mybir.dt.float32

_cache = {}
LAST_IN_MAPS_A = None   # kept for test harness re-runs/profiling
LAST_IN_MAPS_B = None


def _build_phase_a():
    bf16 = mybir.dt.bfloat16
    nc = bacc.Bacc("TRN2", target_bir_lowering=False, debug=False, num_devices=NCORES)
    # x/w pre-split on host into bf16 hi+lo: x = xh + xl exactly to ~2^-17 rel.
    # 3 bf16 matmuls (hh, hl, lh) at full PE rate replace one 1/4-rate fp32
    # matmul; products are exact in fp32, PSUM accumulation identical.
    xTh = nc.dram_tensor("xTh", [D, TOK], bf16, kind="ExternalInput")
    xTl = nc.dram_tensor("xTl", [D, TOK], bf16, kind="ExternalInput")
    wh = nc.dram_tensor("wh", [E, D, C], bf16, kind="ExternalInput")
    wl = nc.dram_tensor("wl", [E, D, C], bf16, kind="ExternalInput")
    linc = nc.dram_tensor("linc", [P, P], f32, kind="ExternalInput")     # linc[k,m]=1 if k<=m
    ones_k1 = nc.dram_tensor("ones_k1", [1, P], f32, kind="ExternalInput")
    ones128 = nc.dram_tensor("ones128", [P, 1], f32, kind="ExternalInput")
    iota8 = nc.dram_tensor("iota8", [P, E], f32, kind="ExternalInput")
    ebias = nc.dram_tensor("ebias", [P, E], f32, kind="ExternalInput")
    probs_out = nc.dram_tensor("probs", [TOK, E], f32, kind="ExternalOutput")
    s0_out = nc.dram_tensor("s0", [TOK, E], f32, kind="ExternalOutput")
    s1_out = nc.dram_tensor("s1", [TOK, E], f32, kind="ExternalOutput")

    with TileContext(nc) as tc:
        with (
            tc.tile_pool(name="const", bufs=1) as cpool,
            tc.tile_pool(name="wbuf", bufs=2) as wpool,
            tc.tile_pool(name="work", bufs=3) as spool,
            tc.tile_pool(name="ss", bufs=1) as sspool,
            tc.tile_pool(name="psum", bufs=8, space="PSUM") as psum,
        ):
            # x^T hi/lo resident in variable k-chunk tiles; W per expert likewise
            # (double-buffered). DMAs are issued in consumption order and the
            # first chunk is a single k-block, so the first matmuls wait on
            # ~0.8MB instead of the full 12MB.
            CHUNKS = [1, 3, 4, 4, 4]           # k-blocks per chunk, sums to KT
            CH0 = [sum(CHUNKS[:i]) for i in range(len(CHUNKS))]  # chunk k-starts
            NCH = len(CHUNKS)

            def _x_chunk(dram, q, name):
                nk = CHUNKS[q]
                tile_ = cpool.tile([P, nk * TOK], bf16, tag=name, name=name)
                nc.sync.dma_start(
                    out=tile_[:].rearrange("p (k n) -> p k n", k=nk),
                    in_=dram.ap()[CH0[q] * P:(CH0[q] + nk) * P, :]
                        .rearrange("(k p) n -> p k n", p=P),
                )
                return tile_

            def _w_chunk(dram, e, q, tag, name):
                nk = CHUNKS[q]
                tile_ = wpool.tile([P, nk * C], bf16, tag=tag, name=name)
                nc.sync.dma_start(
                    out=tile_[:].rearrange("p (k c) -> p k c", k=nk),
                    in_=dram.ap()[e, CH0[q] * P:(CH0[q] + nk) * P, :]
                        .rearrange("(k p) c -> p k c", p=P),
                )
                return tile_

            def _w_expert(e):
                return (
                    [_w_chunk(wh, e, q, f"whq{q}", f"wh{e}_{q}") for q in range(NCH)],
                    [_w_chunk(wl, e, q, f"wlq{q}", f"wl{e}_{q}") for q in range(NCH)],
                )

            # consumption-order issue: W(e0,q0), x(q0), W(e0,q1), x(q1), ...
            wth0_q, wtl0_q = [], []
            xth_q, xtl_q = [], []
            for q in range(NCH):
                wth0_q.append(_w_chunk(wh, 0, q, f"whq{q}", f"wh0_{q}"))
                wtl0_q.append(_w_chunk(wl, 0, q, f"wlq{q}", f"wl0_{q}"))
                xth_q.append(_x_chunk(xTh, q, f"xthq{q}"))
                xtl_q.append(_x_chunk(xTl, q, f"xtlq{q}"))
            linc_sb = cpool.tile([P, P], f32, tag="linc")
            nc.sync.dma_start(out=linc_sb[:], in_=linc.ap()[:, :])
            onesk1_sb = cpool.tile([1, P], f32, tag="onesk1")
            nc.sync.dma_start(out=onesk1_sb[:], in_=ones_k1.ap()[:, :])
            ones128_sb = cpool.tile([P, 1], f32, tag="ones128")
            nc.sync.dma_start(out=ones128_sb[:], in_=ones128.ap()[:, :])
            iota8_sb = cpool.tile([P, E], f32, tag="iota8")
            nc.sync.dma_start(out=iota8_sb[:], in_=iota8.ap()[:, :])
            ebias_sb = cpool.tile([P, E], f32, tag="ebias")
            nc.sync.dma_start(out=ebias_sb[:], in_=ebias.ap()[:, :])
            offs = cpool.tile([1, 2 * E], f32, tag="offs")
            nc.vector.memset(offs[:], 0.0)

            # per-token-tile sum-of-squares accumulators [128, E]
            ss_tiles = [cpool.tile([P, E], f32, tag=f"ss{t}", name=f"ss{t}")
                        for t in range(NT)]

            # ---- matmul phase: for each expert, 8 token tiles x 16 k-tiles ----
            for e in range(E):
                if e == 0:
                    wth_q, wtl_q = wth0_q, wtl0_q
                else:
                    wth_q, wtl_q = _w_expert(e)
                for t in range(NT):
                    ps = psum.tile([P, C], f32, space="PSUM", tag="ps")
                    first = True
                    for k in range(KT):
                        q = max(i for i in range(NCH) if CH0[i] <= k)
                        kq = k - CH0[q]
                        xh_blk = xth_q[q][:, kq * TOK + t * P: kq * TOK + (t + 1) * P]
                        xl_blk = xtl_q[q][:, kq * TOK + t * P: kq * TOK + (t + 1) * P]
                        wh_blk = wth_q[q][:, kq * C:(kq + 1) * C]
                        wl_blk = wtl_q[q][:, kq * C:(kq + 1) * C]
                        for lhsT, rhs in ((xh_blk, wh_blk), (xh_blk, wl_blk), (xl_blk, wh_blk)):
                            nc.tensor.matmul(
                                ps[:], lhsT=lhsT, rhs=rhs,
                                start=first,
                                stop=(k == KT - 1 and rhs is wh_blk and lhsT is xl_blk),
                            )
                            first = False
                    sq = spool.tile([P, C], f32, tag="sq")
                    nc.scalar.activation(sq[:], ps[:], mybir.ActivationFunctionType.Square)
                    red8 = spool.tile([P, 8], f32, tag="red8")
                    nc.vector.tensor_reduce(
                        red8[:], sq[:].rearrange("p (g c) -> p g c", g=8),
                        axis=mybir.AxisListType.X, op=mybir.AluOpType.add,
                    )
                    nc.vector.tensor_reduce(
                        ss_tiles[t][:, e:e + 1], red8[:],
                        axis=mybir.AxisListType.X, op=mybir.AluOpType.add,
                    )

            # ---- routing phase (order matters for the offs chain: t ascending) ----
            for t in range(NT):
                ss = ss_tiles[t]
                # sub-ulp lower-index tie-break: selection on ss - e*1e-4 (~half a
                # logit ulp); softmax shift-invariance keeps probs exact.
                ssb = spool.tile([P, E], f32, tag="ssb")
                nc.vector.tensor_add(out=ssb[:], in0=ss[:], in1=ebias_sb[:])
                top8 = spool.tile([P, 8], f32, tag="top8")
                top8i = spool.tile([P, 8], mybir.dt.uint32, tag="top8i")
                nc.vector.max_with_indices(top8[:], top8i[:], ssb[:])
                idxf = spool.tile([P, 8], f32, tag="idxf")
                nc.vector.tensor_copy(out=idxf[:], in_=top8i[:])

                logits = spool.tile([P, E], f32, tag="logits")
                nc.scalar.activation(logits[:], ss[:], mybir.ActivationFunctionType.Sqrt)
                lmax = spool.tile([P, 1], f32, tag="lmax")
                nc.scalar.activation(lmax[:], top8[:, 0:1], mybir.ActivationFunctionType.Sqrt)
                neg_lmax = spool.tile([P, 1], f32, tag="neglmax")
                nc.vector.tensor_scalar_mul(neg_lmax[:], lmax[:], -1.0)
                expt = spool.tile([P, E], f32, tag="expt")
                nc.scalar.activation(expt[:], logits[:], mybir.ActivationFunctionType.Exp,
                                     bias=neg_lmax[:], scale=1.0)
                denom = spool.tile([P, 1], f32, tag="denom")
                nc.vector.tensor_reduce(denom[:], expt[:], axis=mybir.AxisListType.X,
                                        op=mybir.AluOpType.add)
                rden = spool.tile([P, 1], f32, tag="rden")
                nc.vector.reciprocal(rden[:], denom[:])
                probs = spool.tile([P, E], f32, tag="probs")
                nc.vector.tensor_scalar(probs[:], expt[:], rden[:, 0:1], None,
                                        op0=mybir.AluOpType.mult)
                nc.sync.dma_start(out=probs_out.ap()[t * P:(t + 1) * P, :], in_=probs[:])

                for kk, icol in ((0, 0), (1, 1)):
                    m = spool.tile([P, E], f32, tag=f"m{kk}")
                    nc.vector.tensor_scalar(m[:], iota8_sb[:], idxf[:, icol:icol + 1], None,
                                            op0=mybir.AluOpType.is_equal)
                    cum = psum.tile([P, E], f32, space="PSUM", tag="ps")
                    nc.tensor.matmul(cum[:], lhsT=linc_sb[:], rhs=m[:], start=True, stop=False)
                    nc.tensor.matmul(cum[:], lhsT=onesk1_sb[:], rhs=offs[:, kk * E:(kk + 1) * E],
                                     start=False, stop=True)
                    tot = psum.tile([1, E], f32, space="PSUM", tag="ps")
                    nc.tensor.matmul(tot[:], lhsT=ones128_sb[:], rhs=m[:], start=True, stop=True)
                    nc.vector.tensor_add(out=offs[:, kk * E:(kk + 1) * E],
                                         in0=offs[:, kk * E:(kk + 1) * E], in1=tot[:])
                    s = spool.tile([P, E], f32, tag=f"s{kk}")
                    nc.vector.tensor_sub(out=s[:], in0=cum[:], in1=m[:])
                    nc.vector.tensor_scalar(s[:], s[:], 1.0, None, op0=mybir.AluOpType.add)
                    nc.vector.tensor_mul(out=s[:], in0=s[:], in1=m[:])
                    dst = s0_out if kk == 0 else s1_out
                    nc.sync.dma_start(out=dst.ap()[t * P:(t + 1) * P, :], in_=s[:])
    nc.compile()
    return nc


def _build_phase_b(cap=CAP):
    """Scatter expansion: dispatch/combine have <=2 nonzero (t,e) rows per
    token; build only those 2048 rows each and indirect-scatter them into the
    pre-zeroed outputs (4x fewer bytes + 4x less DVE than a dense write)."""
    import concourse.bass as bass
    i32 = mybir.dt.int32
    NR = 2 * TOK          # (token x choice) rows per core
    NG = NR // P          # 16 scatter groups of 128 rows
    nc = bacc.Bacc("TRN2", target_bir_lowering=False, debug=False, num_devices=NCORES)
    slot = nc.dram_tensor("slot", [NR, 1], f32, kind="ExternalInput")
    prob = nc.dram_tensor("prob", [NR, 1], f32, kind="ExternalInput")
    ridx = nc.dram_tensor("ridx", [NR, 1], i32, kind="ExternalInput")
    iota_cap = nc.dram_tensor("iota_cap", [P, cap], f32, kind="ExternalInput")
    disp = nc.dram_tensor("disp", [TOK * E, cap], f32, kind="ExternalOutput")
    comb = nc.dram_tensor("comb", [TOK * E, cap], f32, kind="ExternalOutput")

    with TileContext(nc) as tc:
        with (
            tc.tile_pool(name="const", bufs=1) as cpool,
            tc.tile_pool(name="work", bufs=4) as spool,
        ):
            iota_sb = cpool.tile([P, cap], f32, tag="iota")
            nc.sync.dma_start(out=iota_sb[:], in_=iota_cap.ap()[:, :])
            # batched scatter inputs: [NR,1] -> [128, NG] (group-major columns)
            sl = cpool.tile([P, NG], f32, tag="sl")
            nc.sync.dma_start(out=sl[:], in_=slot.ap()[:, 0].rearrange("(g p) -> p g", p=P))
            pr = cpool.tile([P, NG], f32, tag="pr")
            nc.sync.dma_start(out=pr[:], in_=prob.ap()[:, 0].rearrange("(g p) -> p g", p=P))
            ri = cpool.tile([P, NG], i32, tag="ri")
            nc.sync.dma_start(out=ri[:], in_=ridx.ap()[:, 0].rearrange("(g p) -> p g", p=P))
            for g in range(NG):
                drow = spool.tile([P, cap], f32, tag="drow")
                nc.vector.tensor_scalar(drow[:], iota_sb[:], sl[:, g:g + 1], None,
                                        op0=mybir.AluOpType.is_equal)
                crow = spool.tile([P, cap], f32, tag="crow")
                nc.vector.tensor_scalar(crow[:], iota_sb[:], sl[:, g:g + 1], pr[:, g:g + 1],
                                        op0=mybir.AluOpType.is_equal,
                                        op1=mybir.AluOpType.mult)
                nc.gpsimd.indirect_dma_start(
                    out=disp.ap()[:, :],
                    out_offset=bass.IndirectOffsetOnAxis(ap=ri[:, g:g + 1], axis=0),
                    in_=drow[:], in_offset=None)
                nc.gpsimd.indirect_dma_start(
                    out=comb.ap()[:, :],
                    out_offset=bass.IndirectOffsetOnAxis(ap=ri[:, g:g + 1], axis=0),
                    in_=crow[:], in_offset=None)
    nc.compile()
    return nc


def _get(name, builder):
    if name not in _cache:
        _cache[name] = builder()
    return _cache[name]


def _split_bf16(a):
    import ml_dtypes
    hi = a.astype(ml_dtypes.bfloat16)
    lo = (a - hi.astype(np.float32)).astype(ml_dtypes.bfloat16)
    return hi, lo


def kernel(token_inputs, bottleneck_weights, expert_capacity):
    x = np.ascontiguousarray(np.asarray(token_inputs, dtype=np.float32)).reshape(B * N, D)
    w = np.ascontiguousarray(np.asarray(bottleneck_weights, dtype=np.float32))
    cap = int(expert_capacity)
    assert cap > 0

    wh, wl = _split_bf16(w)
    core_ids = list(range(NCORES))
    consts = {
        "linc": (np.arange(P)[:, None] <= np.arange(P)[None, :]).astype(np.float32),
        "ones_k1": np.ones((1, P), np.float32),
        "ones128": np.ones((P, 1), np.float32),
        "iota8": np.tile(np.arange(E, dtype=np.float32), (P, 1)),
        "ebias": np.tile(-1e-4 * np.arange(E, dtype=np.float32), (P, 1)),
    }
    in_maps_a = []
    for c in core_ids:
        shard_t = np.ascontiguousarray(x[c * TOK:(c + 1) * TOK].T)   # [2048, 1024]
        xh, xl = _split_bf16(shard_t)
        in_maps_a.append({"xTh": xh, "xTl": xl, "wh": wh, "wl": wl, **consts})

    global LAST_IN_MAPS_A, LAST_IN_MAPS_B
    LAST_IN_MAPS_A = in_maps_a
    nc_a = _get("a", _build_phase_a)
    res_a = run_bass_kernel_spmd(nc_a, in_maps_a, core_ids)

    # ---- host glue: cross-core priority offsets (16 scalars per core pair),
    # then per-(token, choice) slot / prob / target-row tables for the scatter.
    ar = np.arange(TOK)
    in_maps_b = []
    iota_cap = np.tile(np.arange(cap, dtype=np.float32), (P, 1))
    for b in range(B):
        ra, rb = res_a.results[2 * b], res_a.results[2 * b + 1]
        s0a, s1a, s0b, s1b = ra["s0"], ra["s1"], rb["s0"], rb["s1"]
        t0a = (s0a > 0).sum(0).astype(np.float32)   # [E] first-choice counts, first half
        t0b = (s0b > 0).sum(0).astype(np.float32)
        t1a = (s1a > 0).sum(0).astype(np.float32)
        n0 = t0a + t0b                               # total first-choice counts
        for s0, s1, pp, off0, off1 in (
            (s0a, s1a, ra["probs"], np.zeros(E, np.float32), n0),
            (s0b, s1b, rb["probs"], t0a, n0 + t1a),
        ):
            e0 = np.argmax(s0 > 0, axis=1)           # chosen expert per (token, k)
            e1 = np.argmax(s1 > 0, axis=1)
            slot0 = s0[ar, e0] - 1 + off0[e0]        # capacity slot (may be >= CAP)
            slot1 = s1[ar, e1] - 1 + off1[e1]
            in_maps_b.append({
                "slot": np.concatenate([slot0, slot1]).astype(np.float32)[:, None],
                "prob": np.concatenate([pp[ar, e0], pp[ar, e1]]).astype(np.float32)[:, None],
                "ridx": np.concatenate([ar * E + e0, ar * E + e1]).astype(np.int32)[:, None],
                "iota_cap": iota_cap,
            })

    LAST_IN_MAPS_B = in_maps_b
    nc_b = _get(f"b{cap}", lambda: _build_phase_b(cap))
    res_b = run_bass_kernel_spmd(nc_b, in_maps_b, core_ids)

    out = np.empty((2, B, N, E, cap), np.float32)
    for c in core_ids:
        b, h = c // 2, c % 2
        sl = slice(h * TOK, (h + 1) * TOK)
        out[0, b, sl] = res_b.results[c]["disp"].reshape(TOK, E, cap)
        out[1, b, sl] = res_b.results[c]["comb"].reshape(TOK, E, cap)
    return out



# revision 2
# speedup vs baseline: 2.1730x; 2.1730x over previous
"""MoE router (AutonomousRouter) for TRN2, 8 NeuronCores.

Computes reference:
    act    = einsum('bnd,edc->bnec', x, W)          B,N,D,E,C = 4,2048,2048,8,512
    logits = ||act||_2 over c                       [B,N,E]
    probs  = softmax(logits, -1)
    top-2 routing with capacity 640 (priority = order within k-major (choice, token) sequence)
    out    = stack([dispatch, combine])             [2,B,N,E,640] fp32

Sharding: data-parallel over tokens; core i <- tokens [i*1024, (i+1)*1024) of the
flattened [8192] token axis. Weights replicated.

Three device phases:
  A1 (coarse): single fp16 matmul per k-tile -> sum-of-squares ss [TOK, E].
      fp16 logit error is <~2e-3 while decision gaps are almost always larger;
      only tokens whose top1/2/3 logit gaps fall under GAP_T need exactness.
  A2 (exact):  fp16 hi/lo split (3 full-rate matmuls, fp32-grade: ~1e-7 logit
      err, measured on HW) for the <=NP ambiguous tokens, expert e on core e.
  B  (rows):   for each (token, choice) build the dispatch one-hot row and the
      prob-scaled combine row densely as fp16 [2*TOK, 2*cap]; host glue
      scatters rows into the zero output during unsharding (no indirect DMA).
Host glue between phases: softmax/top-2/capacity-cumsum on [8192, 8] scalars.
"""
import numpy as np

import concourse.bacc as bacc
import concourse.mybir as mybir
from concourse.tile import TileContext
from concourse.bass_utils import run_bass_kernel_spmd

P = 128          # partitions
B, N, D, E, C = 4, 2048, 2048, 8, 512
CAP = 640
NCORES = 8
T = B * N
TOK = T // NCORES           # tokens per core = 1024
NT = TOK // P               # token tiles per core = 8
KT = D // P                 # contraction tiles = 16

W_SCALE = 32.0              # keep fp16 weights away from subnormals
LO_SCALE = 4096.0           # 2^12 scaling for fp16 split low parts
GAP_T = 1e-2                # coarse logit-gap ambiguity threshold
NP = 512                    # padded ambiguous-token capacity (4 tiles)

f32 = mybir.dt.float32
f16 = mybir.dt.float16

_cache = {}
LAST_IN_MAPS_A1 = None   # kept for test harness re-runs/profiling
LAST_IN_MAPS_A2 = None
LAST_IN_MAPS_B = None


def _build_a1():
    """Coarse pass: ss[t, e] = sum_c (x[t] @ (32*w[e]))_c^2 in fp16 x fp16."""
    nc = bacc.Bacc("TRN2", target_bir_lowering=False, debug=False, num_devices=NCORES)
    xT = nc.dram_tensor("xT", [D, TOK], f16, kind="ExternalInput")
    w = nc.dram_tensor("w", [E, D, C], f16, kind="ExternalInput")
    ss_out = nc.dram_tensor("ss", [TOK, E], f32, kind="ExternalOutput")

    with TileContext(nc) as tc:
        with (
            tc.tile_pool(name="const", bufs=1) as cpool,
            tc.tile_pool(name="work", bufs=3) as spool,
            tc.tile_pool(name="psum", bufs=8, space="PSUM") as psum,
        ):
            # x^T and all of W live in SBUF (21 MB fp16). DMAs are issued in
            # consumption order; the first chunks are single k-blocks so the
            # first matmuls wait on ~0.8MB instead of 21MB.
            CHUNKS = [1, 3, 4, 4, 4]           # k-blocks per chunk, sums to KT
            CH0 = [sum(CHUNKS[:i]) for i in range(len(CHUNKS))]
            NCH = len(CHUNKS)

            def _x_chunk(q):
                nk = CHUNKS[q]
                t_ = cpool.tile([P, nk * TOK], f16, tag=f"xq{q}", name=f"x{q}")
                nc.sync.dma_start(
                    out=t_[:].rearrange("p (k n) -> p k n", k=nk),
                    in_=xT.ap()[CH0[q] * P:(CH0[q] + nk) * P, :]
                        .rearrange("(k p) n -> p k n", p=P),
                )
                return t_

            def _w_chunk(e, q):
                nk = CHUNKS[q]
                t_ = cpool.tile([P, nk * C], f16, tag=f"w{e}q{q}", name=f"w{e}_{q}")
                nc.sync.dma_start(
                    out=t_[:].rearrange("p (k c) -> p k c", k=nk),
                    in_=w.ap()[e, CH0[q] * P:(CH0[q] + nk) * P, :]
                        .rearrange("(k p) c -> p k c", p=P),
                )
                return t_

            xq, w_sb = [], {}
            for q in range(NCH):
                w_sb[(0, q)] = _w_chunk(0, q)
                xq.append(_x_chunk(q))
            for e in range(1, E):
                for q in range(NCH):
                    w_sb[(e, q)] = _w_chunk(e, q)

            ss_tiles = [cpool.tile([P, E], f32, tag=f"ss{t}", name=f"ss{t}")
                        for t in range(NT)]

            for e in range(E):
                for t in range(NT):
                    ps = psum.tile([P, C], f32, space="PSUM", tag="ps")
                    for k in range(KT):
                        q = max(i for i in range(NCH) if CH0[i] <= k)
                        kq = k - CH0[q]
                        nc.tensor.matmul(
                            ps[:],
                            lhsT=xq[q][:, kq * TOK + t * P: kq * TOK + (t + 1) * P],
                            rhs=w_sb[(e, q)][:, kq * C:(kq + 1) * C],
                            start=(k == 0), stop=(k == KT - 1),
                        )
                    sq = spool.tile([P, C], f32, tag="sq")
                    nc.scalar.activation(sq[:], ps[:], mybir.ActivationFunctionType.Square,
                                         accum_out=ss_tiles[t][:, e:e + 1])
            for t in range(NT):
                nc.sync.dma_start(out=ss_out.ap()[t * P:(t + 1) * P, :], in_=ss_tiles[t][:])
    nc.compile()
    return nc


def _build_a2():
    """Exact pass: fp32-grade sumsq for NP gathered tokens x one expert/core.

    x = xh + xls/LO_SCALE, w = wh + wls/LO_SCALE (all fp16);
    a ~= xh@wh + (xh@wls + xls@wh)/LO_SCALE  (xl*wl term ~2^-22 rel, dropped).
    """
    nc = bacc.Bacc("TRN2", target_bir_lowering=False, debug=False, num_devices=NCORES)
    xh = nc.dram_tensor("xh", [D, NP], f16, kind="ExternalInput")
    xls = nc.dram_tensor("xls", [D, NP], f16, kind="ExternalInput")
    wh = nc.dram_tensor("wh", [D, C], f16, kind="ExternalInput")
    wls = nc.dram_tensor("wls", [D, C], f16, kind="ExternalInput")
    ss_out = nc.dram_tensor("ss", [NP, 1], f32, kind="ExternalOutput")
    NT2 = NP // P

    with TileContext(nc) as tc:
        with (
            tc.tile_pool(name="const", bufs=1) as cpool,
            tc.tile_pool(name="work", bufs=3) as spool,
            tc.tile_pool(name="psum", bufs=8, space="PSUM") as psum,
        ):
            def _load(dram, cols, tag):
                t_ = cpool.tile([P, KT * cols], f16, tag=tag, name=tag)
                nc.sync.dma_start(
                    out=t_[:].rearrange("p (k n) -> p k n", k=KT),
                    in_=dram.ap()[:, :].rearrange("(k p) n -> p k n", p=P),
                )
                return t_

            wh_sb = _load(wh, C, "wh")
            xh_sb = _load(xh, NP, "xh")
            wls_sb = _load(wls, C, "wls")
            xls_sb = _load(xls, NP, "xls")
            ss_sb = cpool.tile([P, NT2], f32, tag="ss")

            for t in range(NT2):
                ps_hh = psum.tile([P, C], f32, space="PSUM", tag="ps")
                ps_cr = psum.tile([P, C], f32, space="PSUM", tag="ps")
                for k in range(KT):
                    xh_blk = xh_sb[:, k * NP + t * P: k * NP + (t + 1) * P]
                    xls_blk = xls_sb[:, k * NP + t * P: k * NP + (t + 1) * P]
                    wh_blk = wh_sb[:, k * C:(k + 1) * C]
                    wls_blk = wls_sb[:, k * C:(k + 1) * C]
                    nc.tensor.matmul(ps_hh[:], lhsT=xh_blk, rhs=wh_blk,
                                     start=(k == 0), stop=(k == KT - 1))
                    nc.tensor.matmul(ps_cr[:], lhsT=xh_blk, rhs=wls_blk,
                                     start=(k == 0), stop=False)
                    nc.tensor.matmul(ps_cr[:], lhsT=xls_blk, rhs=wh_blk,
                                     start=False, stop=(k == KT - 1))
                a = spool.tile([P, C], f32, tag="a")
                nc.vector.tensor_scalar(a[:], ps_cr[:], 1.0 / LO_SCALE, None,
                                        op0=mybir.AluOpType.mult)
                nc.vector.tensor_add(out=a[:], in0=a[:], in1=ps_hh[:])
                sq = spool.tile([P, C], f32, tag="sq")
                nc.scalar.activation(sq[:], a[:], mybir.ActivationFunctionType.Square,
                                     accum_out=ss_sb[:, t:t + 1])
            nc.sync.dma_start(
                out=ss_out.ap()[:, 0].rearrange("(t p) -> p t", p=P), in_=ss_sb[:])
    nc.compile()
    return nc


def _build_b(cap=CAP):
    """Row builder: per (token, choice) the dispatch one-hot row (iota==slot)
    and combine row prob*(iota==slot), dense fp16 [2*TOK, 2*cap]. Host glue
    scatters rows by (token, chosen expert) while unsharding."""
    NR = 2 * TOK
    NG = NR // P
    nc = bacc.Bacc("TRN2", target_bir_lowering=False, debug=False, num_devices=NCORES)
    slot = nc.dram_tensor("slot", [NR, 1], f32, kind="ExternalInput")
    prob = nc.dram_tensor("prob", [NR, 1], f32, kind="ExternalInput")
    iota_cap = nc.dram_tensor("iota_cap", [P, cap], f16, kind="ExternalInput")
    rows = nc.dram_tensor("rows", [NR, 2 * cap], f16, kind="ExternalOutput")

    with TileContext(nc) as tc:
        with (
            tc.tile_pool(name="const", bufs=1) as cpool,
            tc.tile_pool(name="work", bufs=4) as spool,
        ):
            iota_sb = cpool.tile([P, cap], f16, tag="iota")
            nc.sync.dma_start(out=iota_sb[:], in_=iota_cap.ap()[:, :])
            sl = cpool.tile([P, NG], f32, tag="sl")
            nc.sync.dma_start(out=sl[:], in_=slot.ap()[:, 0].rearrange("(g p) -> p g", p=P))
            pr = cpool.tile([P, NG], f32, tag="pr")
            nc.sync.dma_start(out=pr[:], in_=prob.ap()[:, 0].rearrange("(g p) -> p g", p=P))
            for g in range(NG):
                rtile = spool.tile([P, 2 * cap], f16, tag="rt")
                nc.vector.tensor_scalar(rtile[:, :cap], iota_sb[:], sl[:, g:g + 1], None,
                                        op0=mybir.AluOpType.is_equal)
                nc.vector.tensor_scalar(rtile[:, cap:], iota_sb[:], sl[:, g:g + 1],
                                        pr[:, g:g + 1],
                                        op0=mybir.AluOpType.is_equal,
                                        op1=mybir.AluOpType.mult)
                nc.sync.dma_start(out=rows.ap()[g * P:(g + 1) * P, :], in_=rtile[:])
    nc.compile()
    return nc


def _get(name, builder):
    if name not in _cache:
        _cache[name] = builder()
    return _cache[name]


def kernel(token_inputs, bottleneck_weights, expert_capacity):
    global LAST_IN_MAPS_A1, LAST_IN_MAPS_A2, LAST_IN_MAPS_B
    x = np.ascontiguousarray(np.asarray(token_inputs, dtype=np.float32)).reshape(T, D)
    w = np.ascontiguousarray(np.asarray(bottleneck_weights, dtype=np.float32))
    cap = int(expert_capacity)
    assert cap > 0
    core_ids = list(range(NCORES))

    # ---- phase A1: coarse fp16 sum-of-squares ----
    w16 = np.ascontiguousarray((w * W_SCALE).astype(np.float16))
    in_maps_a1 = []
    for c in core_ids:
        xT = np.ascontiguousarray(x[c * TOK:(c + 1) * TOK].T).astype(np.float16)
        in_maps_a1.append({"xT": xT, "w": w16})
    LAST_IN_MAPS_A1 = in_maps_a1
    nc1 = _get("a1", _build_a1)
    res1 = run_bass_kernel_spmd(nc1, in_maps_a1, core_ids)
    ss = np.concatenate([r["ss"] for r in res1.results], 0).astype(np.float64)
    ss /= W_SCALE * W_SCALE
    L = np.sqrt(ss)                                   # coarse logits [T, E]

    # ---- ambiguous tokens: any of the top1/2/3 coarse gaps under GAP_T ----
    l_sorted = np.sort(L, axis=1)[:, ::-1]
    rel_gap = np.minimum(l_sorted[:, 0] - l_sorted[:, 1],
                         l_sorted[:, 1] - l_sorted[:, 2])
    amb = np.flatnonzero(rel_gap < GAP_T)
    namb = len(amb)
    assert namb <= NP, f"ambiguous token overflow: {namb} > {NP}"

    # ---- phase A2: exact sumsq for ambiguous tokens (expert e on core e) ----
    xaT = np.zeros((D, NP), np.float32)
    xaT[:, :namb] = x[amb].T
    xh = xaT.astype(np.float16)
    xls = ((xaT - xh.astype(np.float32)) * LO_SCALE).astype(np.float16)
    in_maps_a2 = []
    for e in range(NCORES):
        we = np.ascontiguousarray(w[e])
        wh = we.astype(np.float16)
        wls = ((we - wh.astype(np.float32)) * LO_SCALE).astype(np.float16)
        in_maps_a2.append({"xh": xh, "xls": xls, "wh": wh, "wls": wls})
    LAST_IN_MAPS_A2 = in_maps_a2
    nc2 = _get("a2", _build_a2)
    res2 = run_bass_kernel_spmd(nc2, in_maps_a2, core_ids)
    if namb:
        ss_ex = np.stack([res2.results[e]["ss"].reshape(-1)[:namb]
                          for e in range(NCORES)], 1).astype(np.float64)
        L[amb] = np.sqrt(ss_ex)

    # ---- host glue: top-2 (stable => lower index on ties, like lax.top_k),
    # softmax probs, capacity priorities over the k-major (choice, token) seq.
    order = np.argsort(-L, axis=1, kind="stable")
    e0, e1 = order[:, 0], order[:, 1]
    m = L.max(1, keepdims=True)
    pexp = np.exp(L - m)
    probs = pexp / pexp.sum(1, keepdims=True)
    slot = np.empty((T, 2), np.int64)
    for b in range(B):
        bsl = slice(b * N, (b + 1) * N)
        seq = np.concatenate([e0[bsl], e1[bsl]])
        onehot = seq[:, None] == np.arange(E)[None, :]
        pri = onehot.cumsum(0) - 1
        pv = pri[np.arange(2 * N), seq]
        slot[bsl, 0] = pv[:N]
        slot[bsl, 1] = pv[N:]

    # ---- phase B: build rows on device ----
    ar = np.arange(T)
    p0 = probs[ar, e0].astype(np.float32)
    p1 = probs[ar, e1].astype(np.float32)
    iota16 = np.tile(np.arange(cap, dtype=np.float16), (P, 1))
    in_maps_b = []
    for c in core_ids:
        tsl = slice(c * TOK, (c + 1) * TOK)
        in_maps_b.append({
            "slot": np.concatenate([slot[tsl, 0], slot[tsl, 1]])
                      .astype(np.float32)[:, None],
            "prob": np.concatenate([p0[tsl], p1[tsl]]).astype(np.float32)[:, None],
            "iota_cap": iota16,
        })
    LAST_IN_MAPS_B = in_maps_b
    nc3 = _get(f"b{cap}", lambda: _build_b(cap))
    res3 = run_bass_kernel_spmd(nc3, in_maps_b, core_ids)

    # ---- unshard: scatter rows into the dense output ----
    out = np.zeros((2, T, E, cap), np.float32)
    for c in core_ids:
        rows = res3.results[c]["rows"]                  # [2*TOK, 2*cap] f16
        toks = np.arange(c * TOK, (c + 1) * TOK)
        for k, ek in ((0, e0), (1, e1)):
            rk = rows[k * TOK:(k + 1) * TOK].astype(np.float32)
            out[0, toks, ek[toks]] = rk[:, :cap]
            out[1, toks, ek[toks]] = rk[:, cap:]
    return out.reshape(2, B, N, E, cap)


# revision 7
# speedup vs baseline: 2.2863x; 1.0521x over previous
"""MoE router (AutonomousRouter) for TRN2, 8 NeuronCores.

Computes reference:
    act    = einsum('bnd,edc->bnec', x, W)          B,N,D,E,C = 4,2048,2048,8,512
    logits = ||act||_2 over c                       [B,N,E]
    probs  = softmax(logits, -1)
    top-2 routing with capacity 640 (priority = order within k-major (choice, token) sequence)
    out    = stack([dispatch, combine])             [2,B,N,E,640] fp32

Sharding: data-parallel over tokens; core i <- tokens [i*1024, (i+1)*1024) of the
flattened [8192] token axis. Weights replicated.

Three device phases:
  A1 (coarse): single fp16 matmul per k-tile -> sum-of-squares ss [TOK, E].
      fp16 logit error is <~2e-3 while decision gaps are almost always larger;
      only tokens whose top1/2/3 logit gaps fall under GAP_T need exactness.
  A2 (exact):  fp16 hi/lo split (3 full-rate matmuls, fp32-grade: ~1e-7 logit
      err, measured on HW) for the <=NP ambiguous tokens, expert e on core e.
  B  (rows):   for each (token, choice) build the dispatch one-hot row and the
      prob-scaled combine row densely as fp16 [2*TOK, 2*cap]; host glue
      scatters rows into the zero output during unsharding (no indirect DMA).
Host glue between phases: softmax/top-2/capacity-cumsum on [8192, 8] scalars.
"""
import numpy as np

import concourse.bacc as bacc
import concourse.mybir as mybir
from concourse.tile import TileContext
from concourse.bass_utils import run_bass_kernel_spmd

P = 128          # partitions
B, N, D, E, C = 4, 2048, 2048, 8, 512
CAP = 640
NCORES = 8
T = B * N
TOK = T // NCORES           # tokens per core = 1024
NT = TOK // P               # token tiles per core = 8
KT = D // P                 # contraction tiles = 16

W_SCALE = 32.0              # keep fp16 weights away from subnormals
LO_SCALE = 4096.0           # 2^12 scaling for fp16 split low parts
GAP_T = 1e-2                # coarse logit-gap ambiguity threshold
NP = 512                    # padded ambiguous-token capacity (4 tiles)

f32 = mybir.dt.float32
f16 = mybir.dt.float16

_cache = {}
LAST_IN_MAPS_A1 = None   # kept for test harness re-runs/profiling
LAST_IN_MAPS_A2 = None
LAST_IN_MAPS_B = None


def _build_a1():
    """Coarse pass: ss[t, e] = sum_c (x[t] @ (32*w[e]))_c^2 in fp16 x fp16."""
    nc = bacc.Bacc("TRN2", target_bir_lowering=False, debug=False, num_devices=NCORES)
    xT = nc.dram_tensor("xT", [D, TOK], f16, kind="ExternalInput")
    w = nc.dram_tensor("w", [E, D, C], f16, kind="ExternalInput")
    ss_out = nc.dram_tensor("ss", [TOK, E], f32, kind="ExternalOutput")

    with TileContext(nc) as tc:
        with (
            tc.tile_pool(name="const", bufs=1) as cpool,
            tc.tile_pool(name="work", bufs=3) as spool,
            tc.tile_pool(name="psum", bufs=8, space="PSUM") as psum,
        ):
            # x^T and all of W live in SBUF (21 MB fp16). DMAs are issued in
            # consumption order; the first chunks are single k-blocks so the
            # first matmuls wait on ~0.8MB instead of 21MB.
            CHUNKS = [1, 3, 4, 4, 4]           # k-blocks per chunk, sums to KT
            CH0 = [sum(CHUNKS[:i]) for i in range(len(CHUNKS))]
            NCH = len(CHUNKS)

            def _x_chunk(q):
                nk = CHUNKS[q]
                t_ = cpool.tile([P, nk * TOK], f16, tag=f"xq{q}", name=f"x{q}")
                nc.sync.dma_start(
                    out=t_[:].rearrange("p (k n) -> p k n", k=nk),
                    in_=xT.ap()[CH0[q] * P:(CH0[q] + nk) * P, :]
                        .rearrange("(k p) n -> p k n", p=P),
                )
                return t_

            def _w_chunk(e, q):
                nk = CHUNKS[q]
                t_ = cpool.tile([P, nk * C], f16, tag=f"w{e}q{q}", name=f"w{e}_{q}")
                nc.sync.dma_start(
                    out=t_[:].rearrange("p (k c) -> p k c", k=nk),
                    in_=w.ap()[e, CH0[q] * P:(CH0[q] + nk) * P, :]
                        .rearrange("(k p) c -> p k c", p=P),
                )
                return t_

            xq, w_sb = [], {}
            for q in range(NCH):
                w_sb[(0, q)] = _w_chunk(0, q)
                xq.append(_x_chunk(q))
            for e in range(1, E):
                for q in range(NCH):
                    w_sb[(e, q)] = _w_chunk(e, q)

            ss_tiles = [cpool.tile([P, E], f32, tag=f"ss{t}", name=f"ss{t}")
                        for t in range(NT)]

            for e in range(E):
                for t in range(NT):
                    ps = psum.tile([P, C], f32, space="PSUM", tag="ps")
                    for k in range(KT):
                        q = max(i for i in range(NCH) if CH0[i] <= k)
                        kq = k - CH0[q]
                        nc.tensor.matmul(
                            ps[:],
                            lhsT=xq[q][:, kq * TOK + t * P: kq * TOK + (t + 1) * P],
                            rhs=w_sb[(e, q)][:, kq * C:(kq + 1) * C],
                            start=(k == 0), stop=(k == KT - 1),
                        )
                    sq = spool.tile([P, C], f32, tag="sq")
                    nc.scalar.activation(sq[:], ps[:], mybir.ActivationFunctionType.Square,
                                         accum_out=ss_tiles[t][:, e:e + 1])
            for t in range(NT):
                nc.sync.dma_start(out=ss_out.ap()[t * P:(t + 1) * P, :], in_=ss_tiles[t][:])
    nc.compile()
    return nc


def _build_a2():
    """Exact pass: fp32-grade sumsq for NP gathered tokens x one expert/core.

    x = xh + xls/LO_SCALE, w = wh + wls/LO_SCALE (all fp16);
    a ~= xh@wh + (xh@wls + xls@wh)/LO_SCALE  (xl*wl term ~2^-22 rel, dropped).
    """
    nc = bacc.Bacc("TRN2", target_bir_lowering=False, debug=False, num_devices=NCORES)
    xh = nc.dram_tensor("xh", [D, NP], f16, kind="ExternalInput")
    xls = nc.dram_tensor("xls", [D, NP], f16, kind="ExternalInput")
    wh = nc.dram_tensor("wh", [D, C], f16, kind="ExternalInput")
    wls = nc.dram_tensor("wls", [D, C], f16, kind="ExternalInput")
    ss_out = nc.dram_tensor("ss", [NP, 1], f32, kind="ExternalOutput")
    NT2 = NP // P

    NCH = 4                      # 4 chunks of 4 k-blocks, consumption order
    NKC = KT // NCH

    with TileContext(nc) as tc:
        with (
            tc.tile_pool(name="const", bufs=1) as cpool,
            tc.tile_pool(name="work", bufs=3) as spool,
            tc.tile_pool(name="psum", bufs=1, space="PSUM") as psum,
        ):
            def _chunk(dram, cols, q, tag):
                t_ = cpool.tile([P, NKC * cols], f16, tag=tag, name=tag)
                nc.sync.dma_start(
                    out=t_[:].rearrange("p (k n) -> p k n", k=NKC),
                    in_=dram.ap()[q * NKC * P:(q + 1) * NKC * P, :]
                        .rearrange("(k p) n -> p k n", p=P),
                )
                return t_

            wh_q, xh_q, wls_q, xls_q = [], [], [], []
            for q in range(NCH):
                wh_q.append(_chunk(wh, C, q, f"wh{q}"))
                xh_q.append(_chunk(xh, NP, q, f"xh{q}"))
                wls_q.append(_chunk(wls, C, q, f"wls{q}"))
                xls_q.append(_chunk(xls, NP, q, f"xls{q}"))
            ss_sb = cpool.tile([P, NT2], f32, tag="ss")

            # k-outer / tile-inner: 8 PSUM banks = 4 tiles x {hh, cross};
            # each DMA chunk is consumed by all tiles before the next chunk.
            ps_hh = [psum.tile([P, C], f32, space="PSUM", tag=f"ph{t}",
                               name=f"ph{t}") for t in range(NT2)]
            ps_cr = [psum.tile([P, C], f32, space="PSUM", tag=f"pc{t}",
                               name=f"pc{t}") for t in range(NT2)]
            for k in range(KT):
                q, kq = k // NKC, k % NKC
                wh_blk = wh_q[q][:, kq * C:(kq + 1) * C]
                wls_blk = wls_q[q][:, kq * C:(kq + 1) * C]
                for t in range(NT2):
                    xh_blk = xh_q[q][:, kq * NP + t * P: kq * NP + (t + 1) * P]
                    xls_blk = xls_q[q][:, kq * NP + t * P: kq * NP + (t + 1) * P]
                    nc.tensor.matmul(ps_hh[t][:], lhsT=xh_blk, rhs=wh_blk,
                                     start=(k == 0), stop=(k == KT - 1))
                    nc.tensor.matmul(ps_cr[t][:], lhsT=xh_blk, rhs=wls_blk,
                                     start=(k == 0), stop=False)
                    nc.tensor.matmul(ps_cr[t][:], lhsT=xls_blk, rhs=wh_blk,
                                     start=False, stop=(k == KT - 1))
            for t in range(NT2):
                a = spool.tile([P, C], f32, tag="a")
                nc.vector.tensor_scalar(a[:], ps_cr[t][:], 1.0 / LO_SCALE, None,
                                        op0=mybir.AluOpType.mult)
                nc.vector.tensor_add(out=a[:], in0=a[:], in1=ps_hh[t][:])
                sq = spool.tile([P, C], f32, tag="sq")
                nc.scalar.activation(sq[:], a[:], mybir.ActivationFunctionType.Square,
                                     accum_out=ss_sb[:, t:t + 1])
            nc.sync.dma_start(
                out=ss_out.ap()[:, 0].rearrange("(t p) -> p t", p=P), in_=ss_sb[:])
    nc.compile()
    return nc


def _build_b(cap=CAP):
    """Row builder: per (token, choice) the combine row prob*(iota==slot),
    dense fp16 [2*TOK, cap]. Host glue scatters rows by (token, chosen
    expert) while unsharding and derives dispatch = (combine != 0), exact
    because top-2 softmax probs (>~1e-2) never round to fp16 zero."""
    NR = 2 * TOK
    NG = NR // P
    nc = bacc.Bacc("TRN2", target_bir_lowering=False, debug=False, num_devices=NCORES)
    slot = nc.dram_tensor("slot", [NR, 1], f32, kind="ExternalInput")
    prob = nc.dram_tensor("prob", [NR, 1], f32, kind="ExternalInput")
    iota_cap = nc.dram_tensor("iota_cap", [P, cap], f16, kind="ExternalInput")
    rows = nc.dram_tensor("rows", [NR, cap], f16, kind="ExternalOutput")

    with TileContext(nc) as tc:
        with (
            tc.tile_pool(name="const", bufs=1) as cpool,
            tc.tile_pool(name="work", bufs=4) as spool,
        ):
            iota_sb = cpool.tile([P, cap], f16, tag="iota")
            nc.sync.dma_start(out=iota_sb[:], in_=iota_cap.ap()[:, :])
            sl = cpool.tile([P, NG], f32, tag="sl")
            nc.sync.dma_start(out=sl[:], in_=slot.ap()[:, 0].rearrange("(g p) -> p g", p=P))
            pr = cpool.tile([P, NG], f32, tag="pr")
            nc.sync.dma_start(out=pr[:], in_=prob.ap()[:, 0].rearrange("(g p) -> p g", p=P))
            for g in range(NG):
                rtile = spool.tile([P, cap], f16, tag="rt")
                nc.vector.tensor_scalar(rtile[:], iota_sb[:], sl[:, g:g + 1],
                                        pr[:, g:g + 1],
                                        op0=mybir.AluOpType.is_equal,
                                        op1=mybir.AluOpType.mult)
                nc.sync.dma_start(out=rows.ap()[g * P:(g + 1) * P, :], in_=rtile[:])
    nc.compile()
    return nc


def _get(name, builder):
    if name not in _cache:
        _cache[name] = builder()
    return _cache[name]


def kernel(token_inputs, bottleneck_weights, expert_capacity):
    global LAST_IN_MAPS_A1, LAST_IN_MAPS_A2, LAST_IN_MAPS_B
    x = np.ascontiguousarray(np.asarray(token_inputs, dtype=np.float32)).reshape(T, D)
    w = np.ascontiguousarray(np.asarray(bottleneck_weights, dtype=np.float32))
    cap = int(expert_capacity)
    assert cap > 0
    core_ids = list(range(NCORES))

    # ---- phase A1: coarse fp16 sum-of-squares ----
    w16 = np.ascontiguousarray((w * W_SCALE).astype(np.float16))
    in_maps_a1 = []
    for c in core_ids:
        xT = np.ascontiguousarray(x[c * TOK:(c + 1) * TOK].T).astype(np.float16)
        in_maps_a1.append({"xT": xT, "w": w16})
    LAST_IN_MAPS_A1 = in_maps_a1
    nc1 = _get("a1", _build_a1)
    res1 = run_bass_kernel_spmd(nc1, in_maps_a1, core_ids)
    ss = np.concatenate([r["ss"] for r in res1.results], 0).astype(np.float64)
    ss /= W_SCALE * W_SCALE
    L = np.sqrt(ss)                                   # coarse logits [T, E]

    # ---- ambiguous tokens: any of the top1/2/3 coarse gaps under GAP_T ----
    l_sorted = np.sort(L, axis=1)[:, ::-1]
    rel_gap = np.minimum(l_sorted[:, 0] - l_sorted[:, 1],
                         l_sorted[:, 1] - l_sorted[:, 2])
    amb = np.flatnonzero(rel_gap < GAP_T)
    namb = len(amb)
    assert namb <= NP, f"ambiguous token overflow: {namb} > {NP}"

    # ---- phase A2: exact sumsq for ambiguous tokens (expert e on core e) ----
    xaT = np.zeros((D, NP), np.float32)
    xaT[:, :namb] = x[amb].T
    xh = xaT.astype(np.float16)
    xls = ((xaT - xh.astype(np.float32)) * LO_SCALE).astype(np.float16)
    in_maps_a2 = []
    for e in range(NCORES):
        we = np.ascontiguousarray(w[e])
        wh = we.astype(np.float16)
        wls = ((we - wh.astype(np.float32)) * LO_SCALE).astype(np.float16)
        in_maps_a2.append({"xh": xh, "xls": xls, "wh": wh, "wls": wls})
    LAST_IN_MAPS_A2 = in_maps_a2
    nc2 = _get("a2", _build_a2)
    res2 = run_bass_kernel_spmd(nc2, in_maps_a2, core_ids)
    if namb:
        ss_ex = np.stack([res2.results[e]["ss"].reshape(-1)[:namb]
                          for e in range(NCORES)], 1).astype(np.float64)
        L[amb] = np.sqrt(ss_ex)

    # ---- host glue: top-2 (stable => lower index on ties, like lax.top_k),
    # softmax probs, capacity priorities over the k-major (choice, token) seq.
    order = np.argsort(-L, axis=1, kind="stable")
    e0, e1 = order[:, 0], order[:, 1]
    m = L.max(1, keepdims=True)
    pexp = np.exp(L - m)
    probs = pexp / pexp.sum(1, keepdims=True)
    slot = np.empty((T, 2), np.int64)
    for b in range(B):
        bsl = slice(b * N, (b + 1) * N)
        seq = np.concatenate([e0[bsl], e1[bsl]])
        onehot = seq[:, None] == np.arange(E)[None, :]
        pri = onehot.cumsum(0) - 1
        pv = pri[np.arange(2 * N), seq]
        slot[bsl, 0] = pv[:N]
        slot[bsl, 1] = pv[N:]

    # ---- phase B: build rows on device ----
    ar = np.arange(T)
    p0 = probs[ar, e0].astype(np.float32)
    p1 = probs[ar, e1].astype(np.float32)
    iota16 = np.tile(np.arange(cap, dtype=np.float16), (P, 1))
    in_maps_b = []
    for c in core_ids:
        tsl = slice(c * TOK, (c + 1) * TOK)
        in_maps_b.append({
            "slot": np.concatenate([slot[tsl, 0], slot[tsl, 1]])
                      .astype(np.float32)[:, None],
            "prob": np.concatenate([p0[tsl], p1[tsl]]).astype(np.float32)[:, None],
            "iota_cap": iota16,
        })
    LAST_IN_MAPS_B = in_maps_b
    nc3 = _get(f"b{cap}", lambda: _build_b(cap))
    res3 = run_bass_kernel_spmd(nc3, in_maps_b, core_ids)

    # ---- unshard: scatter rows into the dense output ----
    out = np.zeros((2, T, E, cap), np.float32)
    for c in core_ids:
        rows = res3.results[c]["rows"]                  # [2*TOK, cap] f16
        toks = np.arange(c * TOK, (c + 1) * TOK)
        for k, ek in ((0, e0), (1, e1)):
            rk = rows[k * TOK:(k + 1) * TOK].astype(np.float32)
            out[0, toks, ek[toks]] = (rk != 0.0).astype(np.float32)
            out[1, toks, ek[toks]] = rk
    return out.reshape(2, B, N, E, cap)


# revision 16
# speedup vs baseline: 2.3925x; 1.0464x over previous
"""MoE router (AutonomousRouter) for TRN2, 8 NeuronCores.

Computes reference:
    act    = einsum('bnd,edc->bnec', x, W)          B,N,D,E,C = 4,2048,2048,8,512
    logits = ||act||_2 over c                       [B,N,E]
    probs  = softmax(logits, -1)
    top-2 routing with capacity 640 (priority = order within k-major (choice, token) sequence)
    out    = stack([dispatch, combine])             [2,B,N,E,640] fp32

Sharding: data-parallel over tokens; core i <- tokens [i*1024, (i+1)*1024) of the
flattened [8192] token axis. Weights replicated.

Three device phases:
  A1 (coarse): single fp16 matmul per k-tile -> sum-of-squares ss [TOK, E].
      fp16 logit error is <~2e-3 while decision gaps are almost always larger;
      only tokens whose top1/2/3 logit gaps fall under GAP_T need exactness.
  A2 (exact):  fp16 hi/lo split (3 full-rate matmuls, fp32-grade: ~1e-7 logit
      err, measured on HW) for the <=NP ambiguous tokens, expert e on core e.
  B  (rows):   for each (token, choice) build the dispatch one-hot row and the
      prob-scaled combine row densely as fp16 [2*TOK, 2*cap]; host glue
      scatters rows into the zero output during unsharding (no indirect DMA).
Host glue between phases: softmax/top-2/capacity-cumsum on [8192, 8] scalars.
"""
import numpy as np

import concourse.bacc as bacc
import concourse.mybir as mybir
from concourse.tile import TileContext
from concourse.bass_utils import run_bass_kernel_spmd

P = 128          # partitions
B, N, D, E, C = 4, 2048, 2048, 8, 512
CAP = 640
NCORES = 8
T = B * N
TOK = T // NCORES           # tokens per core = 1024
NT = TOK // P               # token tiles per core = 8
KT = D // P                 # contraction tiles = 16

W_SCALE = 32.0              # keep fp16 weights away from subnormals
LO_SCALE = 4096.0           # 2^12 scaling for fp16 split low parts
GAP_T = 9e-3                # coarse logit-gap ambiguity threshold
NP_OPTS = (384, 512)        # padded ambiguous-token capacities (3 or 4 tiles)

f32 = mybir.dt.float32
f16 = mybir.dt.float16

_cache = {}
LAST_IN_MAPS_A1 = None   # kept for test harness re-runs/profiling
LAST_IN_MAPS_A2 = None
LAST_IN_MAPS_B = None
LAST_NP = None
LAST_NAMB = None


def _build_a1():
    """Coarse pass: ss[t, e] = sum_c (x[t] @ (32*w[e]))_c^2 in fp16 x fp16."""
    nc = bacc.Bacc("TRN2", target_bir_lowering=False, debug=False, num_devices=NCORES)
    xT = nc.dram_tensor("xT", [D, TOK], f16, kind="ExternalInput")
    w = nc.dram_tensor("w", [E, D, C], f16, kind="ExternalInput")
    ss_out = nc.dram_tensor("ss", [TOK, E], f32, kind="ExternalOutput")

    with TileContext(nc) as tc:
        with (
            tc.tile_pool(name="const", bufs=1) as cpool,
            tc.tile_pool(name="work", bufs=3) as spool,
            tc.tile_pool(name="psum", bufs=8, space="PSUM") as psum,
        ):
            # x^T and all of W live in SBUF (21 MB fp16). DMAs are issued in
            # consumption order; the first chunks are single k-blocks so the
            # first matmuls wait on ~0.8MB instead of 21MB.
            CHUNKS = [1, 3, 4, 4, 4]           # k-blocks per chunk, sums to KT
            CH0 = [sum(CHUNKS[:i]) for i in range(len(CHUNKS))]
            NCH = len(CHUNKS)

            def _x_chunk(q):
                nk = CHUNKS[q]
                t_ = cpool.tile([P, nk * TOK], f16, tag=f"xq{q}", name=f"x{q}")
                nc.sync.dma_start(
                    out=t_[:].rearrange("p (k n) -> p k n", k=nk),
                    in_=xT.ap()[CH0[q] * P:(CH0[q] + nk) * P, :]
                        .rearrange("(k p) n -> p k n", p=P),
                )
                return t_

            def _w_chunk(e, q):
                nk = CHUNKS[q]
                t_ = cpool.tile([P, nk * C], f16, tag=f"w{e}q{q}", name=f"w{e}_{q}")
                nc.sync.dma_start(
                    out=t_[:].rearrange("p (k c) -> p k c", k=nk),
                    in_=w.ap()[e, CH0[q] * P:(CH0[q] + nk) * P, :]
                        .rearrange("(k p) c -> p k c", p=P),
                )
                return t_

            xq, w_sb = [], {}
            for q in range(NCH):
                w_sb[(0, q)] = _w_chunk(0, q)
                xq.append(_x_chunk(q))
            for e in range(1, E):
                for q in range(NCH):
                    w_sb[(e, q)] = _w_chunk(e, q)

            ss_sb = cpool.tile([P, NT * E], f32, tag="ss")

            for e in range(E):
                for t in range(NT):
                    ps = psum.tile([P, C], f32, space="PSUM", tag="ps")
                    for k in range(KT):
                        q = max(i for i in range(NCH) if CH0[i] <= k)
                        kq = k - CH0[q]
                        nc.tensor.matmul(
                            ps[:],
                            lhsT=xq[q][:, kq * TOK + t * P: kq * TOK + (t + 1) * P],
                            rhs=w_sb[(e, q)][:, kq * C:(kq + 1) * C],
                            start=(k == 0), stop=(k == KT - 1),
                        )
                    sq = spool.tile([P, C], f32, tag="sq")
                    nc.scalar.activation(sq[:], ps[:], mybir.ActivationFunctionType.Square,
                                         accum_out=ss_sb[:, t * E + e: t * E + e + 1])
            nc.sync.dma_start(
                out=ss_out.ap()[:, :].rearrange("(t p) e -> p t e", p=P),
                in_=ss_sb[:].rearrange("p (t e) -> p t e", e=E))
    nc.compile()
    return nc


def _build_a2(npad):
    """Exact pass: fp32-grade sumsq for npad gathered tokens x one expert/core.

    x = xh + xls/LO_SCALE, w = wh + wls/LO_SCALE (all fp16);
    a ~= xh@wh + (xh@wls + xls@wh)/LO_SCALE  (xl*wl term ~2^-22 rel, dropped).
    """
    nc = bacc.Bacc("TRN2", target_bir_lowering=False, debug=False, num_devices=NCORES)
    xh = nc.dram_tensor("xh", [D, npad], f16, kind="ExternalInput")
    xls = nc.dram_tensor("xls", [D, npad], f16, kind="ExternalInput")
    wh = nc.dram_tensor("wh", [D, C], f16, kind="ExternalInput")
    wls = nc.dram_tensor("wls", [D, C], f16, kind="ExternalInput")
    ss_out = nc.dram_tensor("ss", [npad, 1], f32, kind="ExternalOutput")
    NT2 = npad // P

    CHUNKS = [1, 3, 4, 4, 4]     # k-blocks per chunk, consumption order
    CH0 = [sum(CHUNKS[:i]) for i in range(len(CHUNKS))]
    NCH = len(CHUNKS)

    with TileContext(nc) as tc:
        with (
            tc.tile_pool(name="const", bufs=1) as cpool,
            tc.tile_pool(name="work", bufs=3) as spool,
            tc.tile_pool(name="psum", bufs=1, space="PSUM") as psum,
        ):
            def _chunk(dram, cols, q, tag):
                nk = CHUNKS[q]
                t_ = cpool.tile([P, nk * cols], f16, tag=tag, name=tag)
                nc.sync.dma_start(
                    out=t_[:].rearrange("p (k n) -> p k n", k=nk),
                    in_=dram.ap()[CH0[q] * P:(CH0[q] + nk) * P, :]
                        .rearrange("(k p) n -> p k n", p=P),
                )
                return t_

            wh_q, xh_q, wls_q, xls_q = [], [], [], []
            for q in range(NCH):
                wh_q.append(_chunk(wh, C, q, f"wh{q}"))
                xh_q.append(_chunk(xh, npad, q, f"xh{q}"))
                wls_q.append(_chunk(wls, C, q, f"wls{q}"))
                xls_q.append(_chunk(xls, npad, q, f"xls{q}"))
            ss_sb = cpool.tile([P, NT2], f32, tag="ss")

            # k-outer / tile-inner: up to 8 PSUM banks = NT2 tiles x {hh, cross};
            # each DMA chunk is consumed by all tiles before the next chunk.
            # The last chunk runs tile-outer so tiles finish staggered and the
            # combine/Square tail overlaps the matmul stream.
            ps_hh = [psum.tile([P, C], f32, space="PSUM", tag=f"ph{t}",
                               name=f"ph{t}") for t in range(NT2)]
            ps_cr = [psum.tile([P, C], f32, space="PSUM", tag=f"pc{t}",
                               name=f"pc{t}") for t in range(NT2)]

            def _mms(t, k):
                q = max(i for i in range(NCH) if CH0[i] <= k)
                kq = k - CH0[q]
                xh_blk = xh_q[q][:, kq * npad + t * P: kq * npad + (t + 1) * P]
                xls_blk = xls_q[q][:, kq * npad + t * P: kq * npad + (t + 1) * P]
                wh_blk = wh_q[q][:, kq * C:(kq + 1) * C]
                wls_blk = wls_q[q][:, kq * C:(kq + 1) * C]
                nc.tensor.matmul(ps_hh[t][:], lhsT=xh_blk, rhs=wh_blk,
                                 start=(k == 0), stop=(k == KT - 1))
                nc.tensor.matmul(ps_cr[t][:], lhsT=xh_blk, rhs=wls_blk,
                                 start=(k == 0), stop=False)
                nc.tensor.matmul(ps_cr[t][:], lhsT=xls_blk, rhs=wh_blk,
                                 start=False, stop=(k == KT - 1))

            def _finish(t):
                a = spool.tile([P, C], f32, tag="a")
                nc.vector.tensor_scalar(a[:], ps_cr[t][:], 1.0 / LO_SCALE, None,
                                        op0=mybir.AluOpType.mult)
                nc.vector.tensor_add(out=a[:], in0=a[:], in1=ps_hh[t][:])
                sq = spool.tile([P, C], f32, tag="sq")
                nc.scalar.activation(sq[:], a[:], mybir.ActivationFunctionType.Square,
                                     accum_out=ss_sb[:, t:t + 1])

            LASTQ = CH0[-1]                     # k-start of the last chunk
            for k in range(LASTQ):
                for t in range(NT2):
                    _mms(t, k)
            for t in range(NT2):
                for k in range(LASTQ, KT):
                    _mms(t, k)
                _finish(t)
            nc.sync.dma_start(
                out=ss_out.ap()[:, 0].rearrange("(t p) -> p t", p=P), in_=ss_sb[:])
    nc.compile()
    return nc


def _build_b(cap=CAP):
    """Row builder: per (token, choice) the combine row prob*(iota==slot),
    dense fp16 [2*TOK, cap]. Host glue scatters rows by (token, chosen
    expert) while unsharding and derives dispatch = (combine != 0), exact
    because top-2 softmax probs (>~1e-2) never round to fp16 zero."""
    NR = 2 * TOK
    NG = NR // P
    nc = bacc.Bacc("TRN2", target_bir_lowering=False, debug=False, num_devices=NCORES)
    slot = nc.dram_tensor("slot", [NR, 1], f32, kind="ExternalInput")
    prob = nc.dram_tensor("prob", [NR, 1], f32, kind="ExternalInput")
    iota_cap = nc.dram_tensor("iota_cap", [P, cap], f16, kind="ExternalInput")
    rows = nc.dram_tensor("rows", [NR, cap], f16, kind="ExternalOutput")

    with TileContext(nc) as tc:
        with (
            tc.tile_pool(name="const", bufs=1) as cpool,
            tc.tile_pool(name="work", bufs=4) as spool,
        ):
            iota_sb = cpool.tile([P, cap], f16, tag="iota")
            nc.sync.dma_start(out=iota_sb[:], in_=iota_cap.ap()[:, :])
            sl = cpool.tile([P, NG], f32, tag="sl")
            nc.sync.dma_start(out=sl[:], in_=slot.ap()[:, 0].rearrange("(g p) -> p g", p=P))
            pr = cpool.tile([P, NG], f32, tag="pr")
            nc.sync.dma_start(out=pr[:], in_=prob.ap()[:, 0].rearrange("(g p) -> p g", p=P))
            GPB = 4                           # groups batched per DMA
            for g0 in range(0, NG, GPB):
                rtile = spool.tile([P, GPB * cap], f16, tag="rt")
                for j in range(GPB):
                    g = g0 + j
                    nc.vector.tensor_scalar(rtile[:, j * cap:(j + 1) * cap],
                                            iota_sb[:], sl[:, g:g + 1],
                                            pr[:, g:g + 1],
                                            op0=mybir.AluOpType.is_equal,
                                            op1=mybir.AluOpType.mult)
                nc.sync.dma_start(
                    out=rows.ap()[g0 * P:(g0 + GPB) * P, :]
                        .rearrange("(g p) c -> p g c", p=P),
                    in_=rtile[:].rearrange("p (g c) -> p g c", g=GPB))
    nc.compile()
    return nc


def _get(name, builder):
    if name not in _cache:
        _cache[name] = builder()
    return _cache[name]


def kernel(token_inputs, bottleneck_weights, expert_capacity):
    global LAST_IN_MAPS_A1, LAST_IN_MAPS_A2, LAST_IN_MAPS_B, LAST_NP, LAST_NAMB
    x = np.ascontiguousarray(np.asarray(token_inputs, dtype=np.float32)).reshape(T, D)
    w = np.ascontiguousarray(np.asarray(bottleneck_weights, dtype=np.float32))
    cap = int(expert_capacity)
    assert cap > 0
    core_ids = list(range(NCORES))

    # ---- phase A1: coarse fp16 sum-of-squares ----
    w16 = np.ascontiguousarray((w * W_SCALE).astype(np.float16))
    in_maps_a1 = []
    for c in core_ids:
        xT = np.ascontiguousarray(x[c * TOK:(c + 1) * TOK].T).astype(np.float16)
        in_maps_a1.append({"xT": xT, "w": w16})
    LAST_IN_MAPS_A1 = in_maps_a1
    nc1 = _get("a1", _build_a1)
    res1 = run_bass_kernel_spmd(nc1, in_maps_a1, core_ids)
    ss = np.concatenate([r["ss"] for r in res1.results], 0).astype(np.float64)
    ss /= W_SCALE * W_SCALE
    L = np.sqrt(ss)                                   # coarse logits [T, E]

    # ---- ambiguous tokens: any of the top1/2/3 coarse gaps under GAP_T ----
    l_sorted = np.sort(L, axis=1)[:, ::-1]
    rel_gap = np.minimum(l_sorted[:, 0] - l_sorted[:, 1],
                         l_sorted[:, 1] - l_sorted[:, 2])
    amb = np.flatnonzero(rel_gap < GAP_T)
    namb = len(amb)
    NP = next((n for n in NP_OPTS if namb <= n), None)
    assert NP is not None, f"ambiguous token overflow: {namb} > {NP_OPTS[-1]}"
    LAST_NP, LAST_NAMB = NP, namb

    # ---- phase A2: exact sumsq for ambiguous tokens (expert e on core e) ----
    xaT = np.zeros((D, NP), np.float32)
    xaT[:, :namb] = x[amb].T
    xh = xaT.astype(np.float16)
    xls = ((xaT - xh.astype(np.float32)) * LO_SCALE).astype(np.float16)
    in_maps_a2 = []
    for e in range(NCORES):
        we = np.ascontiguousarray(w[e])
        wh = we.astype(np.float16)
        wls = ((we - wh.astype(np.float32)) * LO_SCALE).astype(np.float16)
        in_maps_a2.append({"xh": xh, "xls": xls, "wh": wh, "wls": wls})
    LAST_IN_MAPS_A2 = in_maps_a2
    nc2 = _get(f"a2_{NP}", lambda: _build_a2(NP))
    res2 = run_bass_kernel_spmd(nc2, in_maps_a2, core_ids)
    if namb:
        ss_ex = np.stack([res2.results[e]["ss"].reshape(-1)[:namb]
                          for e in range(NCORES)], 1).astype(np.float64)
        L[amb] = np.sqrt(ss_ex)

    # ---- host glue: top-2 (stable => lower index on ties, like lax.top_k),
    # softmax probs, capacity priorities over the k-major (choice, token) seq.
    order = np.argsort(-L, axis=1, kind="stable")
    e0, e1 = order[:, 0], order[:, 1]
    m = L.max(1, keepdims=True)
    pexp = np.exp(L - m)
    probs = pexp / pexp.sum(1, keepdims=True)
    slot = np.empty((T, 2), np.int64)
    for b in range(B):
        bsl = slice(b * N, (b + 1) * N)
        seq = np.concatenate([e0[bsl], e1[bsl]])
        onehot = seq[:, None] == np.arange(E)[None, :]
        pri = onehot.cumsum(0) - 1
        pv = pri[np.arange(2 * N), seq]
        slot[bsl, 0] = pv[:N]
        slot[bsl, 1] = pv[N:]

    # ---- phase B: build rows on device ----
    ar = np.arange(T)
    p0 = probs[ar, e0].astype(np.float32)
    p1 = probs[ar, e1].astype(np.float32)
    iota16 = np.tile(np.arange(cap, dtype=np.float16), (P, 1))
    in_maps_b = []
    for c in core_ids:
        tsl = slice(c * TOK, (c + 1) * TOK)
        in_maps_b.append({
            "slot": np.concatenate([slot[tsl, 0], slot[tsl, 1]])
                      .astype(np.float32)[:, None],
            "prob": np.concatenate([p0[tsl], p1[tsl]]).astype(np.float32)[:, None],
            "iota_cap": iota16,
        })
    LAST_IN_MAPS_B = in_maps_b
    nc3 = _get(f"b{cap}", lambda: _build_b(cap))
    res3 = run_bass_kernel_spmd(nc3, in_maps_b, core_ids)

    # ---- unshard: scatter rows into the dense output ----
    out = np.zeros((2, T, E, cap), np.float32)
    for c in core_ids:
        rows = res3.results[c]["rows"]                  # [2*TOK, cap] f16
        toks = np.arange(c * TOK, (c + 1) * TOK)
        for k, ek in ((0, e0), (1, e1)):
            rk = rows[k * TOK:(k + 1) * TOK].astype(np.float32)
            out[0, toks, ek[toks]] = (rk != 0.0).astype(np.float32)
            out[1, toks, ek[toks]] = rk
    return out.reshape(2, B, N, E, cap)
